# revision 18
# baseline (speedup 1.0000x reference)
"""Trainium2 Bass kernel for nn_Distiller distillation loss bundle.

Sharding: 8 cores. SA loss (the heavy [b,2304,2304] gram/log-softmax part)
is sharded (batch, row-half) -> one core per (b in 4) x (half in 2).
pi loss is sharded (batch, pixel-half) on the same cores; lo/ic are sharded
(batch, tensor); pa is sharded (batch, tensor). Each core emits small
partial-sum tensors; the host sums partials and finishes the tiny [21,21] /
[4,4] normalizations (the "all-reduce the final scalars" step).
"""

import numpy as np
from operator import add as _op_add

# ---------------- hardcoded problem shapes ----------------
B = 4
C3, H3, W3 = 64, 48, 48
M3 = H3 * W3                      # 2304
C4, H4, W4 = 128, 24, 24
CO, HO, WO = 21, 128, 128
HWO = HO * WO                     # 16384
INV_SQRT_M = 1.0 / np.sqrt(np.float32(M3))
SCL = float(np.sqrt(INV_SQRT_M))  # folded into both gram operands
NCHUNK = [(0, 512), (512, 512), (1024, 512), (1536, 512), (2048, 256)]
NCORES = 8

_PROG = {}


def _register_sqdiff_op():
    """out = ((in0 - s0) - in1*s1)^2 ; accum_out = sum(out).  One DVE pass
    for the SA inner loop: in0=XL(psum), s0=L, in1=exp(SL), s1=1/SigmaS."""
    import concourse.dve_ops as dvo
    from concourse.dve_spec import Spec, Src0, Src1, C0, C1, Zero, lower, sq
    from concourse.dve_uop import DveOpSpec

    name = "SQDIFF_AFF_REDUCE_ANT"
    for op in dvo.OPS:
        if op.name == name:
            return op

    def _ref(in0, in1, s0, s1, imm2):
        b = ((in0.astype(np.float32) - s0) - in1.astype(np.float32) * s1) ** 2
        b = b.astype(np.float32)
        return b, b.reshape(b.shape[0], -1).sum(axis=-1, keepdims=True)

    spec = Spec(body=sq((Src0 - C0) - Src1 * C1), accum=_op_add,
                accum_init=Zero, reference=_ref)
    opcode = max(dvo._SUB_OPCODE_FOR_NAME.values()) + 1
    assert opcode < 0x20
    shas = {}
    for ver in ("v3", "v4"):
        try:
            uops = lower(spec, ver=ver)
            shas[ver] = DveOpSpec(name=name, opcode=opcode, uops=uops,
                                  rd1_en=True).sha(ver)
        except Exception:
            pass
    dvo._SUB_OPCODE_FOR_NAME[name] = opcode
    op = dvo.DveOp(name, spec, subdim=False, uops_sha=shas)
    dvo.OPS.append(op)
    dvo.CUSTOM_DVE_SPECS[name] = spec
    return op


def _build_program():
    if "nc" in _PROG:
        return _PROG
    import concourse.bass as bass
    import concourse.tile as tile
    from concourse import mybir
    from contextlib import ExitStack

    AF = mybir.ActivationFunctionType
    ALU = mybir.AluOpType
    f32 = mybir.dt.float32
    bf16 = mybir.dt.bfloat16

    nc = bass.Bass()
    EI, EO = dict(kind="ExternalInput"), dict(kind="ExternalOutput")
    # inputs (per-core)
    d_tf3 = nc.dram_tensor("tf3", [C3 * M3], f32, **EI)          # teacher feat3 flat
    d_sf3 = nc.dram_tensor("sf3", [C3, M3], f32, **EI)           # student feat3
    d_w2 = nc.dram_tensor("w2", [2, 3, C3, 96], f32, **EI)       # paired conv wts [half,dw,i,o]
    d_w1 = nc.dram_tensor("w1", [3, C3, 96], f32, **EI)          # single-shift wts dh=2
    d_bias = nc.dram_tensor("bias", [96, 1], f32, **EI)
    d_dx = nc.dram_tensor("dx", [128, 9], f32, **EI)             # host XL diag (scaled)
    d_pit = nc.dram_tensor("pit", [128, CO, 64], f32, **EI)
    d_pis = nc.dram_tensor("pis", [128, CO, 64], f32, **EI)
    d_lo = nc.dram_tensor("lon", [128, CO, 128], f32, **EI)      # pixT == N1
    d_pa = nc.dram_tensor("pa", [128, 576], f32, **EI)
    d_idf = nc.dram_tensor("idf", [128, 128], f32, **EI)
    d_idb = nc.dram_tensor("idb", [128, 128], bf16, **EI)
    # outputs
    o_acc = nc.dram_tensor("o_acc", [128, 9], f32, **EO)
    o_L = nc.dram_tensor("o_L", [128, 9], f32, **EO)
    o_inv = nc.dram_tensor("o_inv", [128, 9], f32, **EO)
    o_ds = nc.dram_tensor("o_ds", [128, 9], f32, **EO)
    o_pi = nc.dram_tensor("o_pi", [128, 1], f32, **EO)
    o_gic = nc.dram_tensor("o_gic", [CO, CO], f32, **EO)
    o_cross = nc.dram_tensor("o_cross", [CO, CO], f32, **EO)
    o_se = nc.dram_tensor("o_se", [1, 42], f32, **EO)
    o_pa = nc.dram_tensor("o_pa", [4, 4], f32, **EO)

    with tile.TileContext(nc) as tc, ExitStack() as ctx:
        const = ctx.enter_context(tc.tile_pool(name="const", bufs=1))
        big = ctx.enter_context(tc.tile_pool(name="big", bufs=1))
        outs = ctx.enter_context(tc.tile_pool(name="outs", bufs=1))

        idf = const.tile([128, 128], f32)
        nc.sync.dma_start(out=idf, in_=d_idf[:])
        idb = const.tile([128, 128], bf16)
        nc.sync.dma_start(out=idb, in_=d_idb[:])
        bias_sb = const.tile([96, 1], f32)
        nc.sync.dma_start(out=bias_sb, in_=d_bias[:])
        dx_sb = const.tile([128, 9], f32)
        nc.sync.dma_start(out=dx_sb, in_=d_dx[:])
        w2_sb = const.tile([128, 3, 96], f32)
        w2v = d_w2[:].rearrange("x dw i o -> x i dw o")
        nc.sync.dma_start(out=w2_sb[0:64], in_=w2v[0])
        nc.sync.dma_start(out=w2_sb[64:128], in_=w2v[1])
        w1_sb = const.tile([64, 3, 96], f32)
        nc.sync.dma_start(out=w1_sb, in_=d_w1[:].rearrange("dw i o -> i dw o"))

        # result tiles accumulated across phases
        acc_sb = outs.tile([128, 9], f32)
        L_sb = outs.tile([128, 9], f32)
        inv_sb = outs.tile([128, 9], f32)
        ds_sb = outs.tile([128, 9], f32)
        se_sb = outs.tile([1, 42], f32)

        # ---------------- pa: maxpool + raw gram ----------------
        pap = ctx.enter_context(tc.tile_pool(name="pa_p", bufs=1))
        with tc.tile_pool(name="pa_ps", bufs=1, space="PSUM") as paps:
            pa_sb = pap.tile([128, 576], f32)
            nc.sync.dma_start(out=pa_sb, in_=d_pa[:])
            s1 = pap.tile([128, 24, 2], f32)
            nc.vector.tensor_reduce(
                out=s1, in_=pa_sb.rearrange("p (h wb w) -> p h wb w", wb=2, w=12),
                axis=mybir.AxisListType.X, op=ALU.max)
            s2 = pap.tile([128, 2, 2], f32)
            nc.vector.tensor_reduce(
                out=s2, in_=s1.rearrange("p (hb h) wb -> p hb wb h", hb=2),
                axis=mybir.AxisListType.X, op=ALU.max)
            f4 = s2.rearrange("p a b -> p (a b)")
            g_pa = paps.tile([4, 4], f32)
            nc.tensor.matmul(g_pa, f4, f4, start=True, stop=True)
            pa_out = pap.tile([4, 4], f32)
            nc.vector.tensor_copy(pa_out, g_pa)
            nc.gpsimd.dma_start(out=o_pa[:], in_=pa_out)

        # ---------------- pi: softmax-over-c KL pieces ----------------
        pip = ctx.enter_context(tc.tile_pool(name="pi_p", bufs=1))
        if True:
            pit = pip.tile([128, CO, 64], f32)
            nc.sync.dma_start(out=pit, in_=d_pit[:])
            pis = pip.tile([128, CO, 64], f32)
            nc.sync.dma_start(out=pis, in_=d_pis[:])
            et = pip.tile([128, CO, 64], f32)
            nc.scalar.activation(et, pit, AF.Exp)
            es = pip.tile([128, CO, 64], f32)
            nc.scalar.activation(es, pis, AF.Exp)
            st = pip.tile([128, 64], f32)
            nc.vector.tensor_reduce(out=st, in_=et.rearrange("p c f -> p f c"),
                                    axis=mybir.AxisListType.X, op=ALU.add)
            ss = pip.tile([128, 64], f32)
            nc.vector.tensor_reduce(out=ss, in_=es.rearrange("p c f -> p f c"),
                                    axis=mybir.AxisListType.X, op=ALU.add)
            diff = pip.tile([128, CO, 64], f32)
            nc.vector.tensor_tensor(out=diff, in0=pit, in1=pis, op=ALU.subtract)
            prod = pip.tile([128, CO, 64], f32)
            nc.vector.tensor_tensor(out=prod, in0=et, in1=diff, op=ALU.mult)
            sp = pip.tile([128, 64], f32)
            nc.vector.tensor_reduce(out=sp, in_=prod.rearrange("p c f -> p f c"),
                                    axis=mybir.AxisListType.X, op=ALU.add)
            rinv = pip.tile([128, 64], f32)
            nc.vector.reciprocal(rinv, st)
            r1 = pip.tile([128, 64], f32)
            nc.vector.tensor_tensor(out=r1, in0=sp, in1=rinv, op=ALU.mult)
            lt = pip.tile([128, 64], f32)
            nc.scalar.activation(lt, st, AF.Ln)
            ls = pip.tile([128, 64], f32)
            nc.scalar.activation(ls, ss, AF.Ln)
            r2 = pip.tile([128, 64], f32)
            nc.vector.tensor_tensor(out=r2, in0=r1, in1=lt, op=ALU.subtract)
            r3 = pip.tile([128, 64], f32)
            nc.vector.tensor_tensor(out=r3, in0=r2, in1=ls, op=ALU.add)
            pi_out = pip.tile([128, 1], f32)
            nc.vector.tensor_reduce(out=pi_out, in_=r3,
                                    axis=mybir.AxisListType.X, op=ALU.add)
            nc.gpsimd.dma_start(out=o_pi[:], in_=pi_out)

        # ---------------- lo + ic on (b, tensor) unit ----------------
        lop = ctx.enter_context(tc.tile_pool(name="lo_p", bufs=1))
        losc = ctx.enter_context(tc.tile_pool(name="lo_sc", bufs=2))
        with tc.tile_pool(name="lo_ps", bufs=2, space="PSUM") as lops, \
             tc.tile_pool(name="acc_ps", bufs=1, space="PSUM") as accps:
            lo_sb = lop.tile([128, CO, 128], f32)
            nc.sync.dma_start(out=lo_sb, in_=d_lo[:])
            # ic gram over 16384 pixels
            g_ic = accps.tile([CO, 64], f32, tag="gic")
            for f in range(128):
                nc.tensor.matmul(g_ic[:, 0:CO], lo_sb[:, :, f], lo_sb[:, :, f],
                                 start=(f == 0), stop=(f == 127))
            gic_out = lop.tile([CO, CO], f32)
            nc.vector.tensor_copy(gic_out, g_ic[:, 0:CO])
            nc.gpsimd.dma_start(out=o_gic[:], in_=gic_out)
            # lo: per-class transpose -> exp -> sums
            E2 = lop.tile([128, CO, 128], f32)
            M1 = lop.tile([128, CO], f32)
            SE = lop.tile([128, CO], f32)
            for grp in range(7):
                pT = lops.tile([128, 3, 128], f32, tag="pT")
                for j in range(3):
                    c = grp * 3 + j
                    nc.tensor.transpose(pT[:, j, :], lo_sb[:, c, :], idf)
                nc.scalar.activation(E2[:, grp * 3:grp * 3 + 3, :], pT, AF.Exp)
                pr = losc.tile([128, 3, 128], f32, tag="pr")
                nc.vector.tensor_tensor(out=pr, in0=pT,
                                        in1=E2[:, grp * 3:grp * 3 + 3, :],
                                        op=ALU.mult)
                nc.vector.tensor_reduce(out=M1[:, grp * 3:grp * 3 + 3], in_=pr,
                                        axis=mybir.AxisListType.X, op=ALU.add)
                nc.vector.tensor_reduce(out=SE[:, grp * 3:grp * 3 + 3],
                                        in_=E2[:, grp * 3:grp * 3 + 3, :],
                                        axis=mybir.AxisListType.X, op=ALU.add)
            invSE = lop.tile([128, CO], f32)
            nc.vector.reciprocal(invSE, SE)
            T3 = lop.tile([128, CO], f32)
            nc.vector.tensor_tensor(out=T3, in0=M1, in1=invSE, op=ALU.mult)
            logSE = lop.tile([128, CO], f32)
            nc.scalar.activation(logSE, SE, AF.Ln)
            ones = lop.tile([128, 1], f32)
            nc.vector.memset(ones, 1.0)
            se_ps = accps.tile([1, 64], f32, tag="seps")
            nc.tensor.matmul(se_ps[:, 0:CO], ones, T3, start=True, stop=True)
            nc.tensor.matmul(se_ps[:, 32:32 + CO], ones, logSE, start=True, stop=True)
            nc.vector.tensor_copy(se_sb[:, 0:CO], se_ps[:, 0:CO])
            nc.vector.tensor_copy(se_sb[:, CO:2 * CO], se_ps[:, 32:32 + CO])
            nc.gpsimd.dma_start(out=o_se[:], in_=se_sb)
            # normalize P and cross-gram
            P2 = lop.tile([128, CO, 128], f32)
            for c in range(CO):
                nc.vector.tensor_scalar(out=P2[:, c, :], in0=E2[:, c, :],
                                        scalar1=invSE[:, c:c + 1], scalar2=None,
                                        op0=ALU.mult)
            g_cr = accps.tile([CO, 64], f32, tag="crps")
            for f in range(128):
                nc.tensor.matmul(g_cr[:, 0:CO], P2[:, :, f], P2[:, :, f],
                                 start=(f == 0), stop=(f == 127))
            cr_out = lop.tile([CO, CO], f32)
            nc.vector.tensor_copy(cr_out, g_cr[:, 0:CO])
            nc.gpsimd.dma_start(out=o_cross[:], in_=cr_out)

        # ---------------- conv (student feat3, both convs packed) -------------
        Y = big.tile([96, M3], bf16)
        cvp = ctx.enter_context(tc.tile_pool(name="cv_p", bufs=1))
        with tc.tile_pool(name="cv_ps", bufs=2, space="PSUM") as cvps:
            pad2 = cvp.tile([128, 2500], f32)
            nc.vector.memset(pad2, 0.0)
            sf3v = d_sf3[:].rearrange("ch (h w) -> ch h w", h=H3)
            padv = pad2.rearrange("p (h w) -> p h w", h=50)
            nc.sync.dma_start(out=padv[0:64, 1:49, 1:49], in_=sf3v)
            # second copy pre-shifted by +50 (dh=1 lives at same free offsets)
            nc.sync.dma_start(out=padv[64:128, 0:48, 1:49], in_=sf3v)
            for ci in range(6):
                r0 = ci * 8                                   # output row block
                ps = cvps.tile([96, 384], f32, tag="cps")
                for dw in range(3):
                    nc.tensor.matmul(ps, w2_sb[:, dw, :],
                                     padv[:, r0:r0 + 8, dw:dw + 48],
                                     start=(dw == 0), stop=False)
                for dw in range(3):
                    nc.tensor.matmul(ps, w1_sb[:, dw, :],
                                     padv[0:64, r0 + 2:r0 + 10, dw:dw + 48],
                                     start=False, stop=(dw == 2))
                nc.vector.tensor_scalar(
                    out=Y[:, r0 * 48:(r0 + 8) * 48], in0=ps,
                    scalar1=bias_sb[:], scalar2=None, op0=ALU.add)

        # ---------------- reshuffle to TF-block layout + d_S ----------------
        TFB_T = big.tile([128, 18, 64], bf16)
        nc.gpsimd.dma_start(
            out=TFB_T, in_=d_tf3[:].rearrange("(k p c) -> p k c", k=18, p=128))
        TFB_C = big.tile([128, 18, 64], bf16)
        nc.gpsimd.dma_start(
            out=TFB_C, in_=Y[0:64].rearrange("ch (s c) -> ch s c", c=64))
        TFB_B = big.tile([128, 9, 64], bf16)
        nc.gpsimd.dma_start(
            out=TFB_B, in_=Y[64:96].rearrange("ch (s c) -> ch s c", c=64))
        # d_S: conv-C output channels are host-rotated so this core's m-half
        # is always blocks 0..8 of TFB_C (matching TFB_B and the rotated
        # teacher) — local indices work on every core.
        dsp = ctx.enter_context(tc.tile_pool(name="dsp", bufs=2))
        if True:
            for j in range(9):
                scr = dsp.tile([128, 64], f32, tag="dscr")
                nc.vector.tensor_tensor(out=scr, in0=TFB_B[:, j, :],
                                        in1=TFB_C[:, j, :], op=ALU.mult)
                dsr = dsp.tile([128, 1], f32, tag="dsr")
                nc.vector.tensor_reduce(out=dsr, in_=scr,
                                        axis=mybir.AxisListType.X, op=ALU.add)
                nc.vector.tensor_scalar(out=ds_sb[:, j:j + 1], in0=dsr,
                                        scalar1=float(INV_SQRT_M), scalar2=None,
                                        op0=ALU.mult)

        # ---------------- transposes into combined gram operands -------------
        comb = big.tile([128, M3], bf16)      # top: teacher TFT, bottom: TFT_C
        combB = big.tile([128, 1152], bf16)   # bottom half: TFT_B
        with tc.tile_pool(name="tp_ps", bufs=3, space="PSUM") as tpps:
            for k in range(18):
                pt = tpps.tile([128, 128], bf16, tag="tps")
                nc.tensor.transpose(pt[0:64], TFB_T[:, k, :], idb)
                eng = nc.vector if (k % 2 == 0) else nc.scalar
                if eng is nc.vector:
                    nc.vector.tensor_scalar(out=comb[0:64, k * 128:(k + 1) * 128],
                                            in0=pt[0:64], scalar1=SCL, scalar2=None,
                                            op0=ALU.mult)
                else:
                    nc.scalar.mul(comb[0:64, k * 128:(k + 1) * 128], pt[0:64], SCL)
            for k in range(18):
                pt = tpps.tile([128, 128], bf16, tag="tps")
                nc.tensor.transpose(pt[64:128], TFB_C[:, k, :], idb,
                                    tile_position=(0, 64))
                if k % 2 == 0:
                    nc.vector.tensor_scalar(out=comb[64:128, k * 128:(k + 1) * 128],
                                            in0=pt[64:128], scalar1=SCL, scalar2=None,
                                            op0=ALU.mult)
                else:
                    nc.scalar.mul(comb[64:128, k * 128:(k + 1) * 128], pt[64:128], SCL)
            for k in range(9):
                pt = tpps.tile([128, 128], bf16, tag="tps")
                nc.tensor.transpose(pt[64:128], TFB_B[:, k, :], idb,
                                    tile_position=(0, 64))
                if k % 2 == 0:
                    nc.vector.tensor_scalar(out=combB[64:128, k * 128:(k + 1) * 128],
                                            in0=pt[64:128], scalar1=SCL, scalar2=None,
                                            op0=ALU.mult)
                else:
                    nc.scalar.mul(combB[64:128, k * 128:(k + 1) * 128], pt[64:128], SCL)

        # ---------------- SA main loop ----------------
        sasc = ctx.enter_context(tc.tile_pool(name="sa_sc", bufs=2))
        sasm = ctx.enter_context(tc.tile_pool(name="sa_sm", bufs=2))
        with tc.tile_pool(name="sa_px", bufs=1, space="PSUM") as sapx, \
             tc.tile_pool(name="sa_ps", bufs=2, space="PSUM") as saps:
            # m0c column offset of this core's rows inside comb: the host packs
            # teacher columns so that the core's half sits at columns hf*1152;
            # we always use local mi*128 within combB, and teacher lhsT uses
            # column (HF_COL_BASE + mi*128). HF base is baked by the host via
            # d_tf3 ordering: host rotates teacher flat so that the core's half
            # is first. See host prep. So lhsT base = mi*128 always.
            for mi in range(9):
                pX = sapx.tile([128, M3], f32, tag="pX")
                for (n0, nw) in NCHUNK:
                    nc.tensor.matmul(pX[:, n0:n0 + nw],
                                     comb[0:64, mi * 128:(mi + 1) * 128],
                                     comb[0:64, n0:n0 + nw],
                                     start=True, stop=True)
                scrX = sasc.tile([128, M3], bf16, tag="scrX")
                sX = sasm.tile([128, 1], f32, tag="sX")
                nc.scalar.activation(scrX, pX, AF.Exp, accum_out=sX)
                ES = sasc.tile([128, M3], bf16, tag="ES")
                sSp = sasm.tile([128, 5], f32, tag="sSp")
                for ncI, (n0, nw) in enumerate(NCHUNK):
                    pS = saps.tile([128, 512], f32, tag="pS")
                    nc.tensor.matmul(pS[:, 0:nw],
                                     combB[64:128, mi * 128:(mi + 1) * 128],
                                     comb[64:128, n0:n0 + nw],
                                     start=True, stop=True)
                    nc.scalar.activation(ES[:, n0:n0 + nw], pS[:, 0:nw], AF.Exp,
                                         accum_out=sSp[:, ncI:ncI + 1])
                sS = sasm.tile([128, 1], f32, tag="sS")
                nc.vector.tensor_reduce(out=sS, in_=sSp,
                                        axis=mybir.AxisListType.X, op=ALU.add)
                # row scalars
                eDX = sasm.tile([128, 1], f32, tag="eDX")
                nc.scalar.activation(eDX, dx_sb[:, mi:mi + 1], AF.Exp)
                sXc = sasm.tile([128, 1], f32, tag="sXc")
                nc.vector.scalar_tensor_tensor(out=sXc, in0=sX, scalar=1.0,
                                               in1=eDX, op0=ALU.add,
                                               op1=ALU.subtract)
                nc.scalar.activation(L_sb[:, mi:mi + 1], sXc, AF.Ln)
                eDS = sasm.tile([128, 1], f32, tag="eDS")
                nc.scalar.activation(eDS, ds_sb[:, mi:mi + 1], AF.Exp)
                sSc = sasm.tile([128, 1], f32, tag="sSc")
                nc.vector.scalar_tensor_tensor(out=sSc, in0=sS, scalar=1.0,
                                               in1=eDS, op0=ALU.add,
                                               op1=ALU.subtract)
                nc.vector.reciprocal(inv_sb[:, mi:mi + 1], sSc)
                # D pass: t = S - XL (DVE), then Square(t + L) accum (ACT)
                tD = sasc.tile([128, M3], bf16, tag="tD")
                nc.vector.scalar_tensor_tensor(
                    out=tD, in0=ES, scalar=inv_sb[:, mi:mi + 1], in1=pX,
                    op0=ALU.mult, op1=ALU.subtract)
                scrD = sasc.tile([128, M3], bf16, tag="scrD")
                nc.scalar.activation(scrD, tD, AF.Square,
                                     bias=L_sb[:, mi:mi + 1], scale=1.0,
                                     accum_out=acc_sb[:, mi:mi + 1])
        nc.gpsimd.dma_start(out=o_acc[:], in_=acc_sb)
        nc.gpsimd.dma_start(out=o_L[:], in_=L_sb)
        nc.gpsimd.dma_start(out=o_inv[:], in_=inv_sb)
        nc.gpsimd.dma_start(out=o_ds[:], in_=ds_sb)

    _split_multi_waits(nc)
    _PROG["nc"] = nc
    return _PROG


def _split_multi_waits(nc):
    """This toolchain's codegen allows at most ONE embedded sem-wait per
    instruction. Move extra waits onto standalone NoOps on the same engine
    immediately before the instruction (engine streams are in-order, so the
    gating is equivalent)."""
    from concourse import mybir
    for f in nc.m.functions:
        for bb in f.blocks:
            insts = bb.instructions
            out = []
            changed = False
            for inst in insts:
                si = getattr(inst, "sync_info", None)
                if si is not None and si.on_wait and len(si.on_wait) > 1:
                    waits = list(si.on_wait)
                    eng = inst.engine
                    for w in waits[:-1]:
                        nop = mybir.InstNoOp(
                            name=nc.get_next_instruction_name(),
                            ins=[], outs=[],
                            sync_info=mybir.SyncInfo(on_wait=[w], on_update=[]),
                            bass_nofuse=True,
                            engine=eng)
                        out.append(nop)
                    inst.sync_info = mybir.SyncInfo(
                        on_wait=[waits[-1]], on_update=list(si.on_update or []))
                    changed = True
                out.append(inst)
            if changed:
                bb.instructions = out


def _host_prep(inputs):
    """Build the 8 per-core input maps."""
    import ml_dtypes
    t3 = np.ascontiguousarray(inputs["t_feat3"], np.float32)
    s3 = np.ascontiguousarray(inputs["s_feat3"], np.float32)
    t4 = np.ascontiguousarray(inputs["t_feat4"], np.float32)
    s4 = np.ascontiguousarray(inputs["s_feat4"], np.float32)
    to = np.ascontiguousarray(inputs["t_out"], np.float32)
    so = np.ascontiguousarray(inputs["s_out"], np.float32)
    WB, bB = inputs["W_B"].astype(np.float32), inputs["b_B"].astype(np.float32)
    WC, bC = inputs["W_C"].astype(np.float32), inputs["b_C"].astype(np.float32)

    idf = np.eye(128, dtype=np.float32)
    idb = np.eye(128, dtype=ml_dtypes.bfloat16)
    in_maps = []
    for cid in range(NCORES):
        ib, hf = cid // 2, cid % 2
        # teacher flat, rotated so this core's m-half comes first
        tf = t3[ib].reshape(-1)
        tfr = np.concatenate([tf[hf * 73728:(hf + 1) * 73728],
                              tf[(1 - hf) * 73728:(2 - hf) * 73728]]) \
            if hf == 1 else tf
        TF = t3[ib].reshape(M3, C3)
        mrows = TF[hf * 1152:(hf + 1) * 1152]
        dx = ((mrows.astype(np.float64) ** 2).sum(1) * INV_SQRT_M).astype(np.float32)
        dx = dx.reshape(9, 128).T.copy()
        # conv weights: o-packing [C channels (rotated), B channels half]
        # lhsT layout [i, o]. C out-channels rotated by hf*32 so this core's
        # m-half is always TFB_C blocks 0..8 (matches rotated teacher order).
        wc = WC.reshape(64, 64, 3, 3)
        wc = np.concatenate([wc[hf * 32:], wc[:hf * 32]], axis=0)
        wb = WB.reshape(64, 64, 3, 3)[hf * 32:(hf + 1) * 32]
        w2 = np.zeros((2, 3, 64, 96), np.float32)
        w1 = np.zeros((3, 64, 96), np.float32)
        for dw in range(3):
            for dh, half in ((0, 0), (1, 1)):
                w2[half, dw, :, 0:64] = wc[:, :, dh, dw].T
                w2[half, dw, :, 64:96] = wb[:, :, dh, dw].T
            w1[dw, :, 0:64] = wc[:, :, 2, dw].T
            w1[dw, :, 64:96] = wb[:, :, 2, dw].T
        bC_r = np.concatenate([bC[hf * 32:], bC[:hf * 32]])
        bias = np.concatenate([bC_r, bB[hf * 32:(hf + 1) * 32]]).reshape(96, 1)
        # pi tiles
        pit = to[ib].reshape(CO, HWO)[:, hf * 8192:(hf + 1) * 8192] \
            .reshape(CO, 128, 64).transpose(1, 0, 2).copy()
        pis = so[ib].reshape(CO, HWO)[:, hf * 8192:(hf + 1) * 8192] \
            .reshape(CO, 128, 64).transpose(1, 0, 2).copy()
        # lo/ic unit
        lon = (to if hf == 0 else so)[cid // 2].reshape(CO, HO, WO) \
            .transpose(1, 0, 2).copy()
        # pa unit
        pa = (t4 if hf == 0 else s4)[cid // 2].reshape(128, 576).copy()
        in_maps.append({
            "tf3": np.ascontiguousarray(tfr), "sf3": s3[ib].reshape(C3, M3).copy(),
            "w2": w2, "w1": w1, "bias": bias, "dx": dx,
            "pit": pit, "pis": pis, "lon": lon, "pa": pa,
            "idf": idf, "idb": idb,
        })
    return in_maps


def _host_finish(inputs, results):
    t3 = np.asarray(inputs["t_feat3"], np.float32)
    s_out = np.asarray(inputs["s_out"])
    f32 = np.float32

    # SA
    total = 0.0
    for cid in range(NCORES):
        r = results[cid]
        ib, hf = cid // 2, cid % 2
        acc = r["o_acc"].T.reshape(-1).astype(np.float64)   # [9,128]->[1152]
        L = r["o_L"].T.reshape(-1).astype(np.float64)
        inv = r["o_inv"].T.reshape(-1).astype(np.float64)
        dS = r["o_ds"].T.reshape(-1).astype(np.float64)
        TF = t3[ib].reshape(M3, C3)[hf * 1152:(hf + 1) * 1152]
        dX = (TF.astype(np.float64) ** 2).sum(1) * INV_SQRT_M
        computed = ((dX - L) - np.exp(dS) * inv) ** 2
        true_d = (L + inv) ** 2
        total += acc.sum() - computed.sum() + true_d.sum()
    sa = np.sqrt(total) / (B * M3 * M3)

    # pi
    pi_tot = sum(float(results[c]["o_pi"].astype(np.float64).sum())
                 for c in range(NCORES))
    pi = pi_tot / (B * CO * HO * WO)

    # ic
    def rn(G):
        n = np.sqrt((G * G).sum(-1, keepdims=True))
        return G / np.maximum(n, 1e-12)
    ic = 0.0
    for ib in range(B):
        Gt = results[2 * ib]["o_gic"].astype(np.float64)
        Gs = results[2 * ib + 1]["o_gic"].astype(np.float64)
        ic += ((rn(Gs) - rn(Gt)) ** 2).sum()
    ic = ic / (CO * B)

    # lo
    def icc(se, cross):
        idx = np.maximum(np.arange(CO)[:, None], np.arange(CO)[None, :])
        K = (se[idx] - cross) / B
        n = np.sqrt((K * K).sum(1, keepdims=True))
        return K / np.maximum(n, 1e-12)
    se_t = np.zeros(CO); se_s = np.zeros(CO)
    cr_t = np.zeros((CO, CO)); cr_s = np.zeros((CO, CO))
    for cid in range(NCORES):
        r = results[cid]
        se_u = r["o_se"].astype(np.float64).reshape(42)
        se_v = se_u[0:21] - se_u[21:42]
        if cid % 2 == 0:
            se_t += se_v; cr_t += r["o_cross"].astype(np.float64)
        else:
            se_s += se_v; cr_s += r["o_cross"].astype(np.float64)
    lo = float(((icc(se_s, cr_s) - icc(se_t, cr_t)) ** 2).mean() / B)

    # pa
    def sim(G):
        n = np.sqrt(np.diag(G)) + 1e-8
        return G / (n[:, None] * n[None, :])
    pa = 0.0
    for ib in range(B):
        Gt = results[2 * ib]["o_pa"].astype(np.float64)
        Gs = results[2 * ib + 1]["o_pa"].astype(np.float64)
        pa += ((sim(Gt) - sim(Gs)) ** 2).sum()
    pa = pa / ((2 * 2) ** 2) / B

    return (s_out, f32(pa), f32(pi), f32(ic), f32(lo), f32(sa))


def kernel(**inputs):
    from concourse.bass_utils import run_bass_kernel_spmd
    prog = _build_program()
    in_maps = _host_prep(inputs)
    import os, time
    kw = {}
    if os.environ.get("KBENCH_TRACE"):
        kw = dict(trace=True, stitch_traces=False)
    t0 = time.perf_counter()
    res = run_bass_kernel_spmd(prog["nc"], in_maps,
                               core_ids=list(range(NCORES)), **kw)
    _PROG["last_wall_ns"] = (time.perf_counter() - t0) * 1e9
    _PROG["last_res"] = res
    return _host_finish(inputs, res.results)
